# revision 1
# baseline (speedup 1.0000x reference)
"""Trainium2 Bass kernel for LlamaDiffSparseKVAttention.

Sharding: tensor-parallel over the 8 KV heads (core h owns KV head h and
Q heads 4h..4h+3).  Host precomputes the observation-window importance
statistics / quantile thresholds / sparsity masks (tiny fraction of FLOPs).

Each core runs ONE fused phase: q-projection (+RoPE), causal GQA attention
over the sparsified KV, and a contraction-split output projection
(partial = o_head_group @ wo[rows of this head group]) producing a
full-shape [S, HID] partial that the host sums over the 8 cores.  This
avoids any device collective and keeps wo resident in SBUF (each core only
needs its 512-row slice).  All SBUF streams are bf16 (PSUM accumulation is
fp32); the partial output is fp16.

The KV cache is compacted: evicted keys (~20%) are dropped on the host, the
kept keys stay position-sorted, and host-built causal masks cover only the
few boundary tiles per query block (padding keys mask to zero, so no
denominator fix-up is needed).

Loop structure keeps the PE dense: block 0 interleaves the four per-head
q-proj chains with their attention (g-outer) so nothing waits on RoPE; for
blocks 1..3 the previous block's out-projection groups are interleaved
between attention kt-groups as PE filler while the scalar engine runs exp.
The softmax-denominator matmuls (M=1) issue back-to-back into 4 distinct PE
column groups and run concurrently in one PSUM bank.
"""

import math
import numpy as np
import ml_dtypes

import concourse.bass as bass
import concourse.bacc as bacc
import concourse.mybir as mybir
from concourse.tile import TileContext
from concourse.bass_utils import run_bass_kernel_spmd

B, S, HID = 1, 2048, 4096
HQ, HKV, D = 32, 8, 128
G = HQ // HKV
OBS, W, SINK = 128, 32, 2
THETA = 500000.0
TOP_FRAC, MID_SPARSITY, LOW_FRAC = 0.05, 0.7, 0.20
K_KEEP = int(math.ceil((1.0 - MID_SPARSITY) * D))
SCALE = 1.0 / math.sqrt(D)

N_CORES = 8
CORE_IDS = list(range(N_CORES))
QB = 512            # query block
NQB = S // QB       # 4
KT = 128            # key tile
NKT_P = HID // KT   # 32 contraction tiles for projections

BF = mybir.dt.bfloat16
FR = mybir.dt.float32r
F32 = mybir.dt.float32
F16 = mybir.dt.float16


def _rope_np(x):
    # x: [H, S, D]
    half = D // 2
    inv = 1.0 / (THETA ** (np.arange(half, dtype=np.float32) / half))
    ang = np.arange(S, dtype=np.float32)[:, None] * inv[None, :]
    cos = np.concatenate([np.cos(ang), np.cos(ang)], -1).astype(np.float32)
    sin = np.concatenate([np.sin(ang), np.sin(ang)], -1).astype(np.float32)
    x1, x2 = x[..., :half], x[..., half:]
    rot = np.concatenate([-x2, x1], -1)
    return x * cos[None] + rot * sin[None]


def _build_program(nkc, jm0):
    """nkc[b]: number of 128-key tiles processed for query block b.
    jm0[b]: first tile index that needs a causal/pad mask for block b."""
    nc = bacc.Bacc()
    L = nkc[NQB - 1] * KT                      # padded compacted key count
    nm = [nkc[b] - jm0[b] for b in range(NQB)]  # masked tiles per block
    moff = [sum(nm[:b]) for b in range(NQB)]
    nm_total = sum(nm)

    hs_T = nc.dram_tensor("hs_T", [HID, S], BF, kind="ExternalInput")
    wq_h = nc.dram_tensor("wq_h", [HID, G * D], BF, kind="ExternalInput")
    ksp_T = nc.dram_tensor("ksp_T", [D, L], BF, kind="ExternalInput")
    vsp_r = nc.dram_tensor("vsp_r", [KT, (L // KT) * D], BF, kind="ExternalInput")
    cos_T = nc.dram_tensor("cos_T", [D, S], F32, kind="ExternalInput")
    ssin_T = nc.dram_tensor("ssin_T", [D, S], F32, kind="ExternalInput")
    masks = nc.dram_tensor("masks", [KT, nm_total * QB], BF, kind="ExternalInput")
    ones_l = nc.dram_tensor("ones_l", [KT, 1], BF, kind="ExternalInput")
    ones_r = nc.dram_tensor("ones_r", [1, KT], FR, kind="ExternalInput")
    wo_h = nc.dram_tensor("wo_h", [128, G * HID], BF, kind="ExternalInput")
    out_ext = nc.dram_tensor("out", [S, HID], F16, kind="ExternalOutput")

    lp = nc.allow_low_precision(reason="bf16 pipeline is intentional")
    lp.__enter__()
    with TileContext(nc) as tc:
        with (
            tc.tile_pool(name="wq", bufs=1) as wq_pool,
            tc.tile_pool(name="wo", bufs=1) as wo_pool,
            tc.tile_pool(name="kv", bufs=1) as kv_pool,
            tc.tile_pool(name="hst", bufs=1) as hs_pool,
            tc.tile_pool(name="qt", bufs=2) as q_pool,
            tc.tile_pool(name="oscp", bufs=2) as osc_pool,
            tc.tile_pool(name="ekp", bufs=2) as e_pool,
            tc.tile_pool(name="tmp", bufs=2) as tmp_pool,
            tc.tile_pool(name="stg", bufs=3) as st_pool,
            tc.tile_pool(name="acc", bufs=1, space="PSUM") as acc_pool,
            tc.tile_pool(name="rot", bufs=3, space="PSUM") as rot_pool,
            tc.tile_pool(name="psl", bufs=1, space="PSUM") as l_pool,
        ):
            ksp_sb = kv_pool.tile([D, L], BF)
            vsp_sb = kv_pool.tile([KT, (L // KT) * D], BF)
            masks_sb = kv_pool.tile([KT, nm_total * QB], BF)
            onesl_sb = kv_pool.tile([KT, 1], BF)
            onesr_sb = kv_pool.tile([1, KT], FR)
            wo_sb = wo_pool.tile([128, G * HID], BF)
            cos_bt = {}
            ssin_bt = {}

            def load_rope_block(b):
                qs = slice(b * QB, (b + 1) * QB)
                cos_bt[b] = kv_pool.tile([D, QB], F32, tag="cosb", name=f"cosb{b}")
                ssin_bt[b] = kv_pool.tile([D, QB], F32, tag="sinb", name=f"sinb{b}")
                nc.sync.dma_start(out=cos_bt[b], in_=cos_T[:, qs])
                nc.sync.dma_start(out=ssin_bt[b], in_=ssin_T[:, qs])

            # ---- loads ordered so q-proj block 0 starts immediately ----
            wq_sb = wq_pool.tile([128, NKT_P * G * D], BF)
            hst0 = []
            for kt in range(NKT_P):
                nc.sync.dma_start(
                    out=wq_sb[:, kt * G * D:(kt + 1) * G * D],
                    in_=wq_h[kt * 128:(kt + 1) * 128, :],
                )
                ht = hs_pool.tile([128, QB], BF, tag=f"hst{kt}")
                nc.sync.dma_start(out=ht, in_=hs_T[kt * 128:(kt + 1) * 128, 0:QB])
                hst0.append(ht)
                if kt == 3:
                    load_rope_block(0)
                if kt == 8:
                    nc.sync.dma_start(out=onesl_sb, in_=ones_l[:])
                    nc.sync.dma_start(out=onesr_sb, in_=ones_r[:])
                    nc.sync.dma_start(out=ksp_sb, in_=ksp_T[:])
                    nc.sync.dma_start(out=vsp_sb, in_=vsp_r[:])
                if kt == 12:
                    nc.sync.dma_start(
                        out=masks_sb[:, 0:nm[0] * QB],
                        in_=masks[:, 0:nm[0] * QB],
                    )
                if kt == 16:
                    nc.sync.dma_start(
                        out=masks_sb[:, nm[0] * QB:],
                        in_=masks[:, nm[0] * QB:],
                    )

            def load_wo():
                for g in range(G):
                    nc.sync.dma_start(
                        out=wo_sb[:, g * HID:(g + 1) * HID],
                        in_=wo_h[:, g * HID:(g + 1) * HID],
                    )

            osc_prev = None

            def emit_outproj_group(bb, osc, tt, fc, evac_vector):
                ps = rot_pool.tile([128, QB], F32, tag="rot", name=f"po{bb}_{tt}_{fc}")
                for g in range(G):
                    nc.tensor.matmul(
                        out=ps[:],
                        lhsT=osc[g][:, tt * 128:(tt + 1) * 128],
                        rhs=wo_sb[:, g * HID + fc * QB: g * HID + (fc + 1) * QB],
                        start=(g == 0),
                        stop=(g == G - 1),
                    )
                st = st_pool.tile([128, QB], F16, tag="st")
                if evac_vector:
                    nc.vector.tensor_scalar_add(st[:], ps[:], 0.0)
                else:
                    nc.scalar.copy(st[:], ps[:])
                nc.sync.dma_start(
                    out=out_ext[bb * QB + tt * 128: bb * QB + (tt + 1) * 128,
                                fc * QB:(fc + 1) * QB],
                    in_=st[:],
                )

            def emit_s_exp_mask(b, kt, g, qt):
                ps_s = rot_pool.tile([KT, QB], F32, tag="rot", name=f"pss{b}_{kt}_{g}")
                nc.tensor.matmul(
                    out=ps_s[:],
                    lhsT=ksp_sb[:, kt * KT:(kt + 1) * KT],
                    rhs=qt[:],
                    start=True,
                    stop=True,
                )
                ek = e_pool.tile([KT, QB], BF, tag=f"ek{g}")
                nc.scalar.activation(
                    ek[:], ps_s[:],
                    mybir.ActivationFunctionType.Exp, scale=SCALE,
                )
                if kt >= jm0[b]:
                    slot = moff[b] + (kt - jm0[b])
                    nc.vector.tensor_mul(
                        ek[:], ek[:],
                        masks_sb[:, slot * QB:(slot + 1) * QB],
                    )
                return ek

            def emit_l(b, kt, g, ek, ps_l):
                nc.tensor.matmul(
                    out=ps_l[32 * g:32 * g + 1, :],
                    lhsT=onesl_sb[:],
                    rhs=ek[:],
                    start=(kt == 0),
                    stop=(kt == nkc[b] - 1),
                    tile_position=(0, 32 * g),
                    skip_group_check=True,
                )

            def emit_o(b, kt, g, ek, ps_o):
                nc.tensor.matmul(
                    out=ps_o[:],
                    lhsT=vsp_sb[:, kt * D:(kt + 1) * D],
                    rhs=ek[:],
                    start=(kt == 0),
                    stop=(kt == nkc[b] - 1),
                )

            def emit_tail(b, ps_l, ps_o, lfs):
                # broadcast l along partitions (PE), then fast reciprocal.
                osc = []
                for g in range(G):
                    ps_r = rot_pool.tile([128, QB], F32, tag="rot", name=f"psr{b}_{g}")
                    nc.tensor.matmul(
                        out=ps_r[:], lhsT=onesr_sb[:], rhs=lfs[g][:],
                        start=True, stop=True,
                    )
                    rsb = tmp_pool.tile([128, QB], F32, tag="rsb")
                    nc.vector.reciprocal_approx_fast(rsb[:], ps_r[:])
                    ot = osc_pool.tile([D, QB], BF, tag=f"osc{g}")
                    nc.vector.tensor_mul(ot[:], ps_o[g][:], rsb[:])
                    osc.append(ot)
                return osc

            def rope(g, pss, b):
                y1 = tmp_pool.tile([D, QB], F32, tag="y1")
                y2 = tmp_pool.tile([D, QB], F32, tag="y2")
                nc.vector.tensor_mul(y1[:], pss[:], cos_bt[b][:])
                nc.vector.tensor_mul(y2[0:64, :], pss[64:128, :], ssin_bt[b][64:128, :])
                nc.vector.tensor_mul(y2[64:128, :], pss[0:64, :], ssin_bt[b][0:64, :])
                qt = q_pool.tile([D, QB], BF, tag=f"qt{g}")
                nc.vector.tensor_add(qt[:], y1[:], y2[:])
                return qt

            # ================= block 0: g-outer fused q-proj+attention ======
            # The PE stalls at the l-matmul waiting for exp+mask of the SAME
            # kt (in-order execution), so head g+1's q-proj matmuls are
            # emitted BETWEEN the s-matmul and the l-matmul as latency cover.
            def emit_qproj_mm(pss, g, kt, hst_tiles):
                nc.tensor.matmul(
                    out=pss[:],
                    lhsT=wq_sb[:, kt * G * D + g * D: kt * G * D + (g + 1) * D],
                    rhs=hst_tiles[kt][:],
                    start=(kt == 0),
                    stop=(kt == NKT_P - 1),
                )

            ps_l0 = l_pool.tile([128, QB], F32, tag="psl", name="psl0")
            ps_o0 = []
            lfs0 = []
            qT = [None] * G
            pss = acc_pool.tile([128, QB], F32, tag="acc0", name="qps0_0")
            for kt in range(NKT_P):
                emit_qproj_mm(pss, 0, kt, hst0)
            qT[0] = rope(0, pss, 0)
            load_wo()
            for g in range(G):
                ps_o = acc_pool.tile([D, QB], F32, tag=f"acc{g}", name=f"pso0_{g}")
                ps_o0.append(ps_o)
                if g < G - 1:
                    pss = acc_pool.tile([128, QB], F32, tag=f"acc{g + 1}",
                                        name=f"qps0_{g + 1}")
                per_kt = (NKT_P + nkc[0] - 1) // nkc[0]
                for kt in range(nkc[0]):
                    ek = emit_s_exp_mask(0, kt, g, qT[g])
                    if g < G - 1:
                        for ktq in range(kt * per_kt,
                                         min((kt + 1) * per_kt, NKT_P)):
                            emit_qproj_mm(pss, g + 1, ktq, hst0)
                    emit_l(0, kt, g, ek, ps_l0)
                    if kt == nkc[0] - 1:
                        lf = tmp_pool.tile([1, QB], FR, tag=f"lf{g}")
                        nc.scalar.copy(lf[:], ps_l0[32 * g:32 * g + 1, :])
                        lfs0.append(lf)
                    emit_o(0, kt, g, ek, ps_o)
                if g < G - 1:
                    qT[g + 1] = rope(g + 1, pss, 0)
            osc_prev = emit_tail(0, ps_l0, ps_o0, lfs0)

            # ================= blocks 1..3 ==================================
            for b in range(1, NQB):
                load_rope_block(b)
                # q-projection (g-outer; hst resident per block)
                hst = []
                for g in range(G):
                    pss = acc_pool.tile([128, QB], F32, tag=f"acc{g}", name=f"qps{b}_{g}")
                    for kt in range(NKT_P):
                        if g == 0:
                            ht = hs_pool.tile([128, QB], BF, tag=f"hst{kt}")
                            nc.sync.dma_start(
                                out=ht,
                                in_=hs_T[kt * 128:(kt + 1) * 128,
                                         b * QB:(b + 1) * QB],
                            )
                            hst.append(ht)
                        nc.tensor.matmul(
                            out=pss[:],
                            lhsT=wq_sb[:, kt * G * D + g * D: kt * G * D + (g + 1) * D],
                            rhs=hst[kt][:],
                            start=(kt == 0),
                            stop=(kt == NKT_P - 1),
                        )
                    qT[g] = rope(g, pss, b)

                # attention (kt-outer / g-inner) with the previous block's
                # out-projection interleaved as PE filler
                op_groups = [(tt, fc) for tt in range(QB // 128)
                             for fc in range(HID // QB)]
                op_next = 0
                nkt = nkc[b]
                ps_l = l_pool.tile([128, QB], F32, tag="psl", name=f"psl{b}")
                ps_o = [
                    acc_pool.tile([D, QB], F32, tag=f"acc{g}", name=f"pso{b}_{g}")
                    for g in range(G)
                ]
                lfs = []
                for kt in range(nkt):
                    eks = [emit_s_exp_mask(b, kt, g, qT[g]) for g in range(G)]
                    # out-proj filler sits BETWEEN s and l so the PE has work
                    # while exp/mask for this kt complete (in-order engine)
                    n_emit = ((kt + 1) * len(op_groups)) // nkt - op_next
                    for _ in range(n_emit):
                        tt, fc = op_groups[op_next]
                        emit_outproj_group(b - 1, osc_prev, tt, fc,
                                           op_next % 2 == 0)
                        op_next += 1
                    for g in range(G):
                        emit_l(b, kt, g, eks[g], ps_l)
                    if kt == nkt - 1:
                        # denominator snapshot on scalar while PE runs o
                        for g in range(G):
                            lf = tmp_pool.tile([1, QB], FR, tag=f"lf{g}")
                            nc.scalar.copy(lf[:], ps_l[32 * g:32 * g + 1, :])
                            lfs.append(lf)
                    for g in range(G):
                        emit_o(b, kt, g, eks[g], ps_o[g])
                osc_prev = emit_tail(b, ps_l, ps_o, lfs)

            # final block's out-projection (no filler available)
            for tt in range(QB // 128):
                for fc in range(HID // QB):
                    emit_outproj_group(NQB - 1, osc_prev, tt, fc, fc % 2 == 1)

    lp.__exit__(None, None, None)
    nc.compile()
    nc.finalize()
    return nc


_NC_CACHE = {}
_LAST_RESULTS = None


def _host_prep(hidden_states, wq, wk, wv):
    hs = hidden_states.reshape(S, HID).astype(np.float32)
    k = (hs @ wk).reshape(S, HKV, D).transpose(1, 0, 2)  # [8, S, D]
    v = (hs @ wv).reshape(S, HKV, D).transpose(1, 0, 2)
    k = _rope_np(k).astype(np.float32)

    obs_q = (hs[S - OBS:] @ wq).reshape(OBS, HQ, D).transpose(1, 0, 2)  # [32, OBS, D]
    half = D // 2
    inv = 1.0 / (THETA ** (np.arange(half, dtype=np.float32) / half))
    ang = np.arange(S - OBS, S)[:, None].astype(np.float32) * inv[None, :]
    cos = np.concatenate([np.cos(ang), np.cos(ang)], -1).astype(np.float32)
    sin = np.concatenate([np.sin(ang), np.sin(ang)], -1).astype(np.float32)
    oq1, oq2 = obs_q[..., :half], obs_q[..., half:]
    obs_q = obs_q * cos[None] + np.concatenate([-oq2, oq1], -1) * sin[None]

    obs_qg = obs_q.reshape(HKV, G, OBS, D)
    s_obs = np.einsum("hgqd,hkd->hgqk", obs_qg, k, optimize=True) * SCALE
    obs_causal = np.arange(S)[None, :] <= (S - OBS + np.arange(OBS))[:, None]
    s_obs = np.where(obs_causal[None, None], s_obs, -np.inf).astype(np.float32)
    m = s_obs.max(-1, keepdims=True)
    e = np.exp(s_obs - m)
    p = e / e.sum(-1, keepdims=True)
    aw = p.astype(np.float32).mean(1)  # [8, OBS, S]
    counts = np.minimum(OBS, S - np.arange(S)).astype(np.float32)
    imp = aw.sum(1) / counts[None, :]  # [8, S]

    imp_c = imp[:, :S - W].reshape(-1)
    t_high = np.quantile(imp_c, 1.0 - TOP_FRAC)
    t_low = np.quantile(imp_c, LOW_FRAC)
    level = np.where(imp >= t_high, 0, np.where(imp < t_low, 2, 1))
    pos = np.arange(S)
    dense = (pos >= S - W) | (pos < SINK)
    level = np.where(dense[None, :], 0, level)

    def topk_mask(x):
        a = np.abs(x)
        thr = np.sort(a, -1)[..., D - K_KEEP]
        return a >= thr[..., None]

    keep_k = np.where((level == 0)[..., None], True, (level == 1)[..., None] & topk_mask(k))
    keep_v = np.where((level == 0)[..., None], True, (level == 1)[..., None] & topk_mask(v))
    k_sp = (k * keep_k).astype(np.float32)
    v_sp = (v * keep_v).astype(np.float32)
    evicted = level == 2  # [8, S]
    return k_sp, v_sp, evicted


def _bf16(x):
    return np.ascontiguousarray(x).astype(ml_dtypes.bfloat16)


def kernel(hidden_states, wq, wk, wv, wo):
    global _LAST_RESULTS

    hs = hidden_states.reshape(S, HID).astype(np.float32)
    k_sp, v_sp, evicted = _host_prep(hidden_states, wq, wk, wv)

    # ---- compact the KV cache: drop evicted keys, keep position order ----
    kept = [np.where(~evicted[h])[0] for h in range(HKV)]
    cle = np.array([[np.searchsorted(kept[h], (b + 1) * QB) for b in range(NQB)]
                    for h in range(HKV)])            # keys with pos < (b+1)*QB
    cl0 = np.array([[np.searchsorted(kept[h], b * QB, side="right") for b in range(NQB)]
                    for h in range(HKV)])            # keys with pos <= b*QB
    nkc = tuple(int(math.ceil(cle[:, b].max() / KT)) for b in range(NQB))
    jm0 = tuple(int(cl0[:, b].min() // KT) for b in range(NQB))
    nm = [nkc[b] - jm0[b] for b in range(NQB)]
    nm_total = sum(nm)
    L = nkc[NQB - 1] * KT

    key = (nkc, jm0)
    if key not in _NC_CACHE:
        _NC_CACHE.clear()
        _NC_CACHE[key] = _build_program(nkc, jm0)
    nc = _NC_CACHE[key]

    hs_T = _bf16(hs.T)
    half = D // 2
    inv = 1.0 / (THETA ** (np.arange(half, dtype=np.float32) / half))
    ang = np.arange(S, dtype=np.float32)[:, None] * inv[None, :]  # [S, 64]
    cosb = np.cos(ang).astype(np.float32)
    sinb = np.sin(ang).astype(np.float32)
    cos_T = np.ascontiguousarray(np.concatenate([cosb, cosb], 1).T)  # [128, S]
    ssin_T = np.ascontiguousarray(np.concatenate([sinb, -sinb], 1).T)  # [128, S]

    in_maps = []
    qq = np.arange(QB)[None, :]
    pp = np.arange(KT)[:, None]
    for h in range(N_CORES):
        idx = kept[h]
        n_kept = len(idx)
        kc = np.zeros((L, D), np.float32)
        vc = np.zeros((L, D), np.float32)
        kc[:n_kept] = k_sp[h][idx]
        vc[:n_kept] = v_sp[h][idx]
        pos_c = np.full(L, 1 << 30, np.int64)
        pos_c[:n_kept] = idx
        # boundary masks: mask[p, q] = pos_c[tile*KT + p] <= b*QB + q
        mk = np.zeros((KT, nm_total * QB), np.float32)
        slot = 0
        for b in range(NQB):
            for j in range(jm0[b], nkc[b]):
                tile_pos = pos_c[j * KT:(j + 1) * KT][:, None]
                mk[:, slot * QB:(slot + 1) * QB] = (tile_pos <= b * QB + qq)
                slot += 1
        vsp_h = vc.reshape(L // KT, KT, D).transpose(1, 0, 2).reshape(KT, (L // KT) * D)
        wo_hh = wo[h * G * D:(h + 1) * G * D, :].reshape(G, 128, HID)
        wo_hh = wo_hh.transpose(1, 0, 2).reshape(128, G * HID)
        in_maps.append({
            "hs_T": hs_T,
            "wq_h": _bf16(wq[:, h * G * D:(h + 1) * G * D]),
            "ksp_T": _bf16(kc.T),
            "vsp_r": _bf16(vsp_h),
            "cos_T": cos_T,
            "ssin_T": ssin_T,
            "masks": _bf16(mk),
            "ones_l": _bf16(np.ones((KT, 1), np.float32)),
            "ones_r": np.ones((1, KT), np.float32),
            "wo_h": _bf16(wo_hh),
        })

    res = run_bass_kernel_spmd(nc, in_maps, CORE_IDS)
    _LAST_RESULTS = res
    acc = res.results[0]["out"].astype(np.float32)
    for i in range(1, N_CORES):
        acc += res.results[i]["out"].astype(np.float32)
    return acc.reshape(B, S, HID)



# revision 10
# speedup vs baseline: 1.0190x; 1.0190x over previous
"""Trainium2 Bass kernel for LlamaDiffSparseKVAttention.

Sharding: tensor-parallel over the 8 KV heads (core h owns KV head h and
Q heads 4h..4h+3).  Host precomputes the observation-window importance
statistics / quantile thresholds / sparsity masks (tiny fraction of FLOPs).

Each core runs ONE fused phase: q-projection (+RoPE), causal GQA attention
over the sparsified KV, and a contraction-split output projection
(partial = o_head_group @ wo[rows of this head group]) producing a
full-shape [S, HID] partial that the host sums over the 8 cores.

Pipeline structure (v2): a single global software pipeline.
 - Phase Q0: block-0 q-proj, kt-outer / g-inner, paced by chunked DMA loads
   of wq+hs (2 contraction tiles per DMA).  g3's chain is half deferred
   into attention filler so the PE/DMA rates balance.
 - Attention for block b runs g-OUTER (one PSUM accumulator bank at a
   time); the o/l matmuls lag the s matmul by one iteration so the
   exp+mask chain is never on the PE critical path.  Softmax denominators
   accumulate in one shared PSUM bank via PE column groups.
 - A unified filler queue (q-proj chains for block b+1, deferred
   out-projection groups of completed blocks) is drained at a uniform
   credit rate inside every attention iteration, keeping the PE dense.
 - hs tiles for block b+1 prefetch in 8 chunked DMAs at block start;
   output stores are batched to [128, HID] staging tiles and issued on
   the gpsimd queue so the sync queue never blocks input prefetches.
"""

import math
from collections import deque
from functools import partial
import numpy as np
import ml_dtypes

import concourse.bass as bass
import concourse.bacc as bacc
import concourse.mybir as mybir
from concourse.tile import TileContext
from concourse.bass_utils import run_bass_kernel_spmd

B, S, HID = 1, 2048, 4096
HQ, HKV, D = 32, 8, 128
G = HQ // HKV
OBS, W, SINK = 128, 32, 2
THETA = 500000.0
TOP_FRAC, MID_SPARSITY, LOW_FRAC = 0.05, 0.7, 0.20
K_KEEP = int(math.ceil((1.0 - MID_SPARSITY) * D))
SCALE = 1.0 / math.sqrt(D)

N_CORES = 8
CORE_IDS = list(range(N_CORES))
QB = 512            # query block
NQB = S // QB       # 4
KT = 128            # key tile
NKT_P = HID // KT   # 32 contraction tiles for projections

BF = mybir.dt.bfloat16
FR = mybir.dt.float32r
F32 = mybir.dt.float32
F16 = mybir.dt.float16


def _rope_np(x):
    # x: [H, S, D]
    half = D // 2
    inv = 1.0 / (THETA ** (np.arange(half, dtype=np.float32) / half))
    ang = np.arange(S, dtype=np.float32)[:, None] * inv[None, :]
    cos = np.concatenate([np.cos(ang), np.cos(ang)], -1).astype(np.float32)
    sin = np.concatenate([np.sin(ang), np.sin(ang)], -1).astype(np.float32)
    x1, x2 = x[..., :half], x[..., half:]
    rot = np.concatenate([-x2, x1], -1)
    return x * cos[None] + rot * sin[None]


def _build_program(nkc, jm0):
    """nkc[b]: number of 128-key tiles processed for query block b.
    jm0[b]: first tile index that needs a causal/pad mask for block b."""
    nc = bacc.Bacc()
    L = nkc[NQB - 1] * KT                      # padded compacted key count
    nm = [nkc[b] - jm0[b] for b in range(NQB)]  # masked tiles per block
    moff = [sum(nm[:b]) for b in range(NQB)]
    nm_total = sum(nm)

    hs_T = nc.dram_tensor("hs_T", [HID, S], BF, kind="ExternalInput")
    wq_h = nc.dram_tensor("wq_h", [HID, G * D], BF, kind="ExternalInput")
    ksp_T = nc.dram_tensor("ksp_T", [D, L], BF, kind="ExternalInput")
    vsp_r = nc.dram_tensor("vsp_r", [KT, (L // KT) * D], BF, kind="ExternalInput")
    cos_T = nc.dram_tensor("cos_T", [D, S], F32, kind="ExternalInput")
    ssin_T = nc.dram_tensor("ssin_T", [D, S], F32, kind="ExternalInput")
    masks = nc.dram_tensor("masks", [KT, nm_total * QB], BF, kind="ExternalInput")
    ones_l = nc.dram_tensor("ones_l", [KT, 1], BF, kind="ExternalInput")
    ones_r = nc.dram_tensor("ones_r", [1, KT], FR, kind="ExternalInput")
    wo_h = nc.dram_tensor("wo_h", [128, G * HID], BF, kind="ExternalInput")
    out_ext = nc.dram_tensor("out", [S, HID], F16, kind="ExternalOutput")

    lp = nc.allow_low_precision(reason="bf16 pipeline is intentional")
    lp.__enter__()
    with TileContext(nc) as tc:
        with (
            tc.tile_pool(name="wq", bufs=1) as wq_pool,
            tc.tile_pool(name="wo", bufs=1) as wo_pool,
            tc.tile_pool(name="kv", bufs=1) as kv_pool,
            tc.tile_pool(name="hst", bufs=1) as hs_pool,
            tc.tile_pool(name="qt", bufs=2) as q_pool,
            tc.tile_pool(name="oscp", bufs=3) as osc_pool,
            tc.tile_pool(name="ekp", bufs=3) as e_pool,
            tc.tile_pool(name="tmp", bufs=2) as tmp_pool,
            tc.tile_pool(name="stg", bufs=2) as st_pool,
            tc.tile_pool(name="acc", bufs=1, space="PSUM") as acc_pool,
            tc.tile_pool(name="qps", bufs=1, space="PSUM") as qps_pool,
            tc.tile_pool(name="rot", bufs=3, space="PSUM") as rot_pool,
            tc.tile_pool(name="psl", bufs=1, space="PSUM") as l_pool,
        ):
            ksp_sb = kv_pool.tile([D, L], BF)
            vsp_sb = kv_pool.tile([KT, (L // KT) * D], BF)
            masks_sb = kv_pool.tile([KT, nm_total * QB], BF)
            onesl_sb = kv_pool.tile([KT, 1], BF)
            onesr_sb = kv_pool.tile([1, KT], FR)
            wq_sb = wq_pool.tile([128, NKT_P * G * D], BF)
            wo_sb = wo_pool.tile([128, G * HID], BF)
            cos_bt = {}
            ssin_bt = {}
            hstb = {}
            qT = {}
            osc = {}

            def load_rope_block(b):
                qs = slice(b * QB, (b + 1) * QB)
                cos_bt[b] = q_pool.tile([D, QB], F32, tag="cosb", name=f"cosb{b}")
                ssin_bt[b] = q_pool.tile([D, QB], F32, tag="sinb", name=f"sinb{b}")
                nc.sync.dma_start(out=cos_bt[b], in_=cos_T[:, qs])
                nc.sync.dma_start(out=ssin_bt[b], in_=ssin_T[:, qs])

            def load_wq_chunk(c, kpc):
                # kpc contraction tiles per chunk
                r0 = c * kpc * 128
                src = wq_h[r0:r0 + kpc * 128, :].rearrange('(a p) d -> p a d', a=kpc)
                dst = wq_sb[:, c * kpc * G * D:(c + 1) * kpc * G * D]
                dst = dst.rearrange('p (a d) -> p a d', a=kpc)
                nc.sync.dma_start(out=dst, in_=src)

            def alloc_hstb(b):
                hstb[b] = hs_pool.tile([128, NKT_P * QB], BF, tag="hstb",
                                       name=f"hstb{b}")

            def load_hst_chunk(b, c, kpc):
                r0 = c * kpc * 128
                qs = slice(b * QB, (b + 1) * QB)
                src = hs_T[r0:r0 + kpc * 128, qs].rearrange('(a p) q -> p a q', a=kpc)
                dst = hstb[b][:, c * kpc * QB:(c + 1) * kpc * QB]
                dst = dst.rearrange('p (a q) -> p a q', a=kpc)
                nc.sync.dma_start(out=dst, in_=src)

            def load_wo():
                for g in range(G):
                    nc.sync.dma_start(
                        out=wo_sb[:, g * HID:(g + 1) * HID],
                        in_=wo_h[:, g * HID:(g + 1) * HID],
                    )

            # ---------------- emission helpers ----------------
            def emit_qproj_mm(pss, b, g, kt):
                nc.tensor.matmul(
                    out=pss[:],
                    lhsT=wq_sb[:, kt * G * D + g * D: kt * G * D + (g + 1) * D],
                    rhs=hstb[b][:, kt * QB:(kt + 1) * QB],
                    start=(kt == 0),
                    stop=(kt == NKT_P - 1),
                )

            def rope_y1(b, g, pss, tmps):
                y1 = tmp_pool.tile([D, QB], F32, tag="y1")
                nc.vector.tensor_mul(y1[:], pss[:], cos_bt[b][:])
                tmps['y1'] = y1

            def rope_y2(b, g, pss, tmps):
                y2 = tmp_pool.tile([D, QB], F32, tag="y2")
                nc.vector.tensor_mul(y2[0:64, :], pss[64:128, :],
                                     ssin_bt[b][64:128, :])
                nc.vector.tensor_mul(y2[64:128, :], pss[0:64, :],
                                     ssin_bt[b][0:64, :])
                tmps['y2'] = y2

            def rope_add(b, g, tmps):
                qt = q_pool.tile([D, QB], BF, tag=f"qt{g}", name=f"qt{b}_{g}")
                nc.vector.tensor_add(qt[:], tmps['y1'][:], tmps['y2'][:])
                qT[(b, g)] = qt

            def emit_s_exp_mask(b, kt, g):
                ps_s = rot_pool.tile([KT, QB], F32, tag="rot", name=f"pss{b}_{kt}_{g}")
                nc.tensor.matmul(
                    out=ps_s[:],
                    lhsT=ksp_sb[:, kt * KT:(kt + 1) * KT],
                    rhs=qT[(b, g)][:],
                    start=True,
                    stop=True,
                )
                ek = e_pool.tile([KT, QB], BF, tag="ek", name=f"ek{b}_{kt}_{g}")
                nc.scalar.activation(
                    ek[:], ps_s[:],
                    mybir.ActivationFunctionType.Exp, scale=SCALE,
                )
                if kt >= jm0[b]:
                    slot = moff[b] + (kt - jm0[b])
                    nc.vector.tensor_mul(
                        ek[:], ek[:],
                        masks_sb[:, slot * QB:(slot + 1) * QB],
                    )
                return ek

            def emit_l(b, kt, g, ek, ps_l):
                nc.tensor.matmul(
                    out=ps_l[32 * g:32 * g + 1, :],
                    lhsT=onesl_sb[:],
                    rhs=ek[:],
                    start=(kt == 0),
                    stop=(kt == nkc[b] - 1),
                    tile_position=(0, 32 * g),
                    skip_group_check=True,
                )

            def emit_o(b, kt, g, ek, ps_o):
                nc.tensor.matmul(
                    out=ps_o[:],
                    lhsT=vsp_sb[:, kt * D:(kt + 1) * D],
                    rhs=ek[:],
                    start=(kt == 0),
                    stop=(kt == nkc[b] - 1),
                )

            # ------------- out-projection (deferred groups) -------------
            st_tiles = {}
            st_count = {}
            evac_ctr = [0]

            def emit_op_group(bb, tt, fc):
                key = (bb, tt)
                if key not in st_tiles:
                    st_tiles[key] = st_pool.tile([128, HID], F16, tag="st",
                                                 name=f"st{bb}_{tt}")
                    st_count[key] = 0
                st = st_tiles[key]
                ps = rot_pool.tile([128, QB], F32, tag="rot", name=f"po{bb}_{tt}_{fc}")
                for g in range(G):
                    nc.tensor.matmul(
                        out=ps[:],
                        lhsT=osc[(bb, g)][:, tt * 128:(tt + 1) * 128],
                        rhs=wo_sb[:, g * HID + fc * QB: g * HID + (fc + 1) * QB],
                        start=(g == 0),
                        stop=(g == G - 1),
                    )
                # evac: 2/3 vector, 1/3 scalar (scalar also runs the exps)
                if evac_ctr[0] % 3 == 2:
                    nc.scalar.copy(st[:, fc * QB:(fc + 1) * QB], ps[:])
                else:
                    nc.vector.tensor_scalar_add(st[:, fc * QB:(fc + 1) * QB],
                                                ps[:], 0.0)
                evac_ctr[0] += 1
                st_count[key] += 1
                if st_count[key] == HID // QB:
                    r0 = bb * QB + tt * 128
                    nc.gpsimd.dma_start(out=out_ext[r0:r0 + 128, :], in_=st[:])
                    del st_tiles[key]

            # ---------------- filler queue ----------------
            # unit = (cost, fn, kind); kind 'q' = q-proj chain work with an
            # end-of-block deadline, 'op' = elastic out-projection work
            units = deque()
            carry = [0.0]

            def pump(credits):
                carry[0] += credits
                while units and carry[0] > 1e-9:
                    cost, fn, _ = units.popleft()
                    carry[0] -= cost
                    fn()

            def pump_all():
                while units:
                    units.popleft()[1]()
                carry[0] = 0.0

            def drain_q_units():
                # force-emit all pending deadline units (preserving their
                # relative order); elastic op units stay queued
                rest = [u for u in units if u[2] == 'op']
                for u in units:
                    if u[2] == 'q':
                        u[1]()
                units.clear()
                units.extend(rest)
                carry[0] = 0.0

            def qchain_units(b, g, kts, pss_holder):
                out = []

                def first(kt=kts[0]):
                    if pss_holder.get('t') is None:
                        pss_holder['t'] = qps_pool.tile(
                            [128, QB], F32, tag="qps", name=f"qps{b}_{g}")
                    emit_qproj_mm(pss_holder['t'], b, g, kt)

                out.append((1.0, first, 'q'))
                for kt in kts[1:]:
                    out.append((1.0, partial(
                        lambda kt_: emit_qproj_mm(pss_holder['t'], b, g, kt_),
                        kt), 'q'))
                return out

            def chain_with_rope(b, g, kts):
                holder = {}
                tmps = {}
                us = qchain_units(b, g, kts, holder)
                us.append((2.0, lambda: rope_y1(b, g, holder['t'], tmps), 'q'))
                us.append((2.0, lambda: rope_y2(b, g, holder['t'], tmps), 'q'))
                us.append((2.0, lambda: rope_add(b, g, tmps), 'q'))
                return us

            def weave(a_units, b_units):
                # proportional merge preserving relative order
                ca = sum(u[0] for u in a_units)
                cb = sum(u[0] for u in b_units)
                out = []
                ia = ib = 0
                sa = sb = 0.0
                while ia < len(a_units) or ib < len(b_units):
                    if ib >= len(b_units):
                        out.append(a_units[ia]); sa += a_units[ia][0]; ia += 1
                    elif ia >= len(a_units):
                        out.append(b_units[ib]); sb += b_units[ib][0]; ib += 1
                    elif sa * cb <= sb * ca:
                        out.append(a_units[ia]); sa += a_units[ia][0]; ia += 1
                    else:
                        out.append(b_units[ib]); sb += b_units[ib][0]; ib += 1
                return out

            # ================= phase Q0: block-0 q-projection =============
            KPC = 2   # contraction tiles per startup DMA chunk
            accq = {}
            for g in range(3):
                accq[g] = acc_pool.tile([128, QB], F32, tag=f"acc{g}",
                                        name=f"q0ps{g}")
            g3_holder = {}
            g3_first_half = list(range(0, NKT_P, 2))   # g3 kts emitted in Q0
            g3_rest = [kt for kt in range(NKT_P) if kt % 2 != 0]

            load_rope_block(0)
            alloc_hstb(0)
            for c in range(NKT_P // KPC):
                load_wq_chunk(c, KPC)
                load_hst_chunk(0, c, KPC)
                if c == 2:
                    nc.sync.dma_start(out=onesl_sb, in_=ones_l[:])
                    nc.sync.dma_start(out=onesr_sb, in_=ones_r[:])
                    nc.sync.dma_start(out=ksp_sb, in_=ksp_T[:])
                if c == 6:
                    # only block 0's mask slice is needed at attn0 start
                    nc.sync.dma_start(out=masks_sb[:, 0:nm[0] * QB],
                                      in_=masks[:, 0:nm[0] * QB])
                if c == 9:
                    nc.sync.dma_start(out=vsp_sb, in_=vsp_r[:])
                if c == 11:
                    load_rope_block(1)
                if c == 13:
                    nc.sync.dma_start(out=masks_sb[:, nm[0] * QB:],
                                      in_=masks[:, nm[0] * QB:])
                for a in range(KPC):
                    kt = c * KPC + a
                    for g in range(3):
                        emit_qproj_mm(accq[g], 0, g, kt)
                    if kt % 2 == 0:
                        if g3_holder.get('t') is None:
                            g3_holder['t'] = qps_pool.tile(
                                [128, QB], F32, tag="qps", name="qps0_3")
                        emit_qproj_mm(g3_holder['t'], 0, 3, kt)
            # ropes for g0..2 inline; g3 finishes as filler
            tmps0 = {}
            rope_y1(0, 0, accq[0], tmps0)
            rope_y2(0, 0, accq[0], tmps0)
            rope_add(0, 0, tmps0)

            def mk_q0_g(g):
                tmps = {}
                return [
                    (2.0, lambda: rope_y1(0, g, accq[g], tmps), 'q'),
                    (2.0, lambda: rope_y2(0, g, accq[g], tmps), 'q'),
                    (2.0, lambda: rope_add(0, g, tmps), 'q'),
                ]

            g3_units = qchain_units(0, 3, g3_rest, g3_holder)
            tmps3 = {}
            g3_units.append((2.0, lambda: rope_y1(0, 3, g3_holder['t'], tmps3), 'q'))
            g3_units.append((2.0, lambda: rope_y2(0, 3, g3_holder['t'], tmps3), 'q'))
            g3_units.append((2.0, lambda: rope_add(0, 3, tmps3), 'q'))
            # PE-only g3 mms first (they cover the rope(0,0) latency), ropes
            # for g1/g2 woven in early so attn0's g1/g2 scaffolds are ready
            pre = g3_units[:6] + mk_q0_g(1) + g3_units[6:10] + mk_q0_g(2) \
                + g3_units[10:]
            for u in pre:
                units.append(u)

            # =================== unified attention pipeline ===============
            q_credits = 4 * (NKT_P + 6)
            n_iters_123 = 4 * (nkc[1] + nkc[2] + nkc[3])
            r_rate = (2 * q_credits + 3 * 4 * (QB // 128) * (HID // QB)) \
                / n_iters_123

            for b in range(NQB):
                nkt = nkc[b]
                n_it = 4 * nkt
                if b >= 1:
                    # all chains reading hstb(b) must be emitted before the
                    # hstb(b+1) DMA below reuses the slot (deadlock otherwise);
                    # this also guarantees qT(b,*) exist before the scaffold
                    drain_q_units()
                if b + 1 < NQB:
                    # prefetch next block inputs; queue next q-proj chains
                    alloc_hstb(b + 1)
                    for c in range(8):
                        load_hst_chunk(b + 1, c, NKT_P // 8)
                    if b == 0:
                        load_wo()
                    if b + 2 < NQB:
                        load_rope_block(b + 2)
                    new_units = []
                    for g in range(G):
                        new_units += chain_with_rope(b + 1, g, list(range(NKT_P)))
                    if b == 0:
                        # keep g3/rope prologue strictly first at block 0
                        units.extend(new_units)
                    else:
                        # weave the chains into only the head of the op queue
                        # so they finish within this block
                        q_cr = sum(u[0] for u in new_units)
                        budget = r_rate * n_it
                        old = list(units)
                        head_cr = max(0.0, budget - q_cr)
                        head = []
                        acc_cr = 0.0
                        while old and acc_cr < head_cr:
                            u = old.pop(0)
                            head.append(u)
                            acc_cr += u[0]
                        merged = weave(new_units, head) + old
                        units.clear()
                        units.extend(merged)
                if b == 0:
                    per_iter = (sum(u[0] for u in units)) / n_it
                else:
                    per_iter = r_rate

                ps_l = l_pool.tile([128, QB], F32, tag="psl", name=f"psl{b}")
                for g in range(G):
                    pso = acc_pool.tile([D, QB], F32, tag=f"acc{g % 3}",
                                        name=f"pso{b}_{g}")
                    prev_ek = None
                    if b == 0 and g == 0:
                        pump(10.0)   # cover rope(0,0) latency with g3/q1 mms
                    for kt in range(nkt):
                        ek = emit_s_exp_mask(b, kt, g)
                        pump(per_iter)
                        if prev_ek is not None:
                            emit_o(b, kt - 1, g, prev_ek, pso)
                            emit_l(b, kt - 1, g, prev_ek, ps_l)
                        prev_ek = ek
                    pump(1.0)
                    emit_o(b, nkt - 1, g, prev_ek, pso)
                    emit_l(b, nkt - 1, g, prev_ek, ps_l)
                    lf = tmp_pool.tile([1, QB], FR, tag=f"lf{g}")
                    nc.scalar.copy(lf[:], ps_l[32 * g:32 * g + 1, :])
                    pump(3.0)
                    ps_r = rot_pool.tile([128, QB], F32, tag="rot",
                                         name=f"psr{b}_{g}")
                    nc.tensor.matmul(
                        out=ps_r[:], lhsT=onesr_sb[:], rhs=lf[:],
                        start=True, stop=True,
                    )
                    rsb = tmp_pool.tile([128, QB], F32, tag="rsb")
                    nc.vector.reciprocal_approx_fast(rsb[:], ps_r[:])
                    ot = osc_pool.tile([D, QB], BF, tag=f"osc{g}",
                                       name=f"osc{b}_{g}")
                    nc.vector.tensor_mul(ot[:], pso[:], rsb[:])
                    osc[(b, g)] = ot

                # this block's out-projection becomes filler for later blocks
                for tt in range(QB // 128):
                    for fc in range(HID // QB):
                        units.append((4.0, partial(emit_op_group, b, tt, fc),
                                      'op'))

            pump_all()

    lp.__exit__(None, None, None)
    nc.compile()
    nc.finalize()
    return nc


_NC_CACHE = {}
_LAST_RESULTS = None


def _host_prep(hidden_states, wq, wk, wv):
    hs = hidden_states.reshape(S, HID).astype(np.float32)
    k = (hs @ wk).reshape(S, HKV, D).transpose(1, 0, 2)  # [8, S, D]
    v = (hs @ wv).reshape(S, HKV, D).transpose(1, 0, 2)
    k = _rope_np(k).astype(np.float32)

    obs_q = (hs[S - OBS:] @ wq).reshape(OBS, HQ, D).transpose(1, 0, 2)  # [32, OBS, D]
    half = D // 2
    inv = 1.0 / (THETA ** (np.arange(half, dtype=np.float32) / half))
    ang = np.arange(S - OBS, S)[:, None].astype(np.float32) * inv[None, :]
    cos = np.concatenate([np.cos(ang), np.cos(ang)], -1).astype(np.float32)
    sin = np.concatenate([np.sin(ang), np.sin(ang)], -1).astype(np.float32)
    oq1, oq2 = obs_q[..., :half], obs_q[..., half:]
    obs_q = obs_q * cos[None] + np.concatenate([-oq2, oq1], -1) * sin[None]

    obs_qg = obs_q.reshape(HKV, G, OBS, D)
    s_obs = np.einsum("hgqd,hkd->hgqk", obs_qg, k, optimize=True) * SCALE
    obs_causal = np.arange(S)[None, :] <= (S - OBS + np.arange(OBS))[:, None]
    s_obs = np.where(obs_causal[None, None], s_obs, -np.inf).astype(np.float32)
    m = s_obs.max(-1, keepdims=True)
    e = np.exp(s_obs - m)
    p = e / e.sum(-1, keepdims=True)
    aw = p.astype(np.float32).mean(1)  # [8, OBS, S]
    counts = np.minimum(OBS, S - np.arange(S)).astype(np.float32)
    imp = aw.sum(1) / counts[None, :]  # [8, S]

    imp_c = imp[:, :S - W].reshape(-1)
    t_high = np.quantile(imp_c, 1.0 - TOP_FRAC)
    t_low = np.quantile(imp_c, LOW_FRAC)
    level = np.where(imp >= t_high, 0, np.where(imp < t_low, 2, 1))
    pos = np.arange(S)
    dense = (pos >= S - W) | (pos < SINK)
    level = np.where(dense[None, :], 0, level)

    def topk_mask(x):
        a = np.abs(x)
        thr = np.sort(a, -1)[..., D - K_KEEP]
        return a >= thr[..., None]

    keep_k = np.where((level == 0)[..., None], True, (level == 1)[..., None] & topk_mask(k))
    keep_v = np.where((level == 0)[..., None], True, (level == 1)[..., None] & topk_mask(v))
    k_sp = (k * keep_k).astype(np.float32)
    v_sp = (v * keep_v).astype(np.float32)
    evicted = level == 2  # [8, S]
    return k_sp, v_sp, evicted


def _bf16(x):
    return np.ascontiguousarray(x).astype(ml_dtypes.bfloat16)


def kernel(hidden_states, wq, wk, wv, wo):
    global _LAST_RESULTS

    hs = hidden_states.reshape(S, HID).astype(np.float32)
    k_sp, v_sp, evicted = _host_prep(hidden_states, wq, wk, wv)

    # ---- compact the KV cache: drop evicted keys, keep position order ----
    kept = [np.where(~evicted[h])[0] for h in range(HKV)]
    cle = np.array([[np.searchsorted(kept[h], (b + 1) * QB) for b in range(NQB)]
                    for h in range(HKV)])            # keys with pos < (b+1)*QB
    cl0 = np.array([[np.searchsorted(kept[h], b * QB, side="right") for b in range(NQB)]
                    for h in range(HKV)])            # keys with pos <= b*QB
    nkc = tuple(int(math.ceil(cle[:, b].max() / KT)) for b in range(NQB))
    jm0 = tuple(int(cl0[:, b].min() // KT) for b in range(NQB))
    nm = [nkc[b] - jm0[b] for b in range(NQB)]
    nm_total = sum(nm)
    L = nkc[NQB - 1] * KT

    key = (nkc, jm0)
    if key not in _NC_CACHE:
        _NC_CACHE.clear()
        _NC_CACHE[key] = _build_program(nkc, jm0)
    nc = _NC_CACHE[key]

    hs_T = _bf16(hs.T)
    half = D // 2
    inv = 1.0 / (THETA ** (np.arange(half, dtype=np.float32) / half))
    ang = np.arange(S, dtype=np.float32)[:, None] * inv[None, :]  # [S, 64]
    cosb = np.cos(ang).astype(np.float32)
    sinb = np.sin(ang).astype(np.float32)
    cos_T = np.ascontiguousarray(np.concatenate([cosb, cosb], 1).T)  # [128, S]
    ssin_T = np.ascontiguousarray(np.concatenate([sinb, -sinb], 1).T)  # [128, S]

    in_maps = []
    qq = np.arange(QB)[None, :]
    for h in range(N_CORES):
        idx = kept[h]
        n_kept = len(idx)
        kc = np.zeros((L, D), np.float32)
        vc = np.zeros((L, D), np.float32)
        kc[:n_kept] = k_sp[h][idx]
        vc[:n_kept] = v_sp[h][idx]
        pos_c = np.full(L, 1 << 30, np.int64)
        pos_c[:n_kept] = idx
        # boundary masks: mask[p, q] = pos_c[tile*KT + p] <= b*QB + q
        mk = np.zeros((KT, nm_total * QB), np.float32)
        slot = 0
        for b in range(NQB):
            for j in range(jm0[b], nkc[b]):
                tile_pos = pos_c[j * KT:(j + 1) * KT][:, None]
                mk[:, slot * QB:(slot + 1) * QB] = (tile_pos <= b * QB + qq)
                slot += 1
        vsp_h = vc.reshape(L // KT, KT, D).transpose(1, 0, 2).reshape(KT, (L // KT) * D)
        wo_hh = wo[h * G * D:(h + 1) * G * D, :].reshape(G, 128, HID)
        wo_hh = wo_hh.transpose(1, 0, 2).reshape(128, G * HID)
        in_maps.append({
            "hs_T": hs_T,
            "wq_h": _bf16(wq[:, h * G * D:(h + 1) * G * D]),
            "ksp_T": _bf16(kc.T),
            "vsp_r": _bf16(vsp_h),
            "cos_T": cos_T,
            "ssin_T": ssin_T,
            "masks": _bf16(mk),
            "ones_l": _bf16(np.ones((KT, 1), np.float32)),
            "ones_r": np.ones((1, KT), np.float32),
            "wo_h": _bf16(wo_hh),
        })

    res = run_bass_kernel_spmd(nc, in_maps, CORE_IDS)
    _LAST_RESULTS = res
    acc = res.results[0]["out"].astype(np.float32)
    for i in range(1, N_CORES):
        acc += res.results[i]["out"].astype(np.float32)
    return acc.reshape(B, S, HID)


# revision 20
# speedup vs baseline: 1.1505x; 1.1291x over previous
"""Trainium2 Bass kernel for LlamaDiffSparseKVAttention.

Sharding: tensor-parallel over the 8 KV heads (core h owns KV head h and
Q heads 4h..4h+3).  Host precomputes the observation-window importance
statistics / quantile thresholds / sparsity masks (tiny fraction of FLOPs).

Each core runs ONE fused phase: q-projection (+RoPE), causal GQA attention
over the sparsified KV, and a contraction-split output projection
(partial = o_head_group @ wo[rows of this head group]) producing a
full-shape [S, HID] partial that the host sums over the 8 cores.

Pipeline structure (v2): a single global software pipeline.
 - Phase Q0: block-0 q-proj, kt-outer / g-inner, paced by chunked DMA loads
   of wq+hs (2 contraction tiles per DMA).  g3's chain is half deferred
   into attention filler so the PE/DMA rates balance.
 - Attention for block b runs g-OUTER (one PSUM accumulator bank at a
   time); the o/l matmuls lag the s matmul by one iteration so the
   exp+mask chain is never on the PE critical path.  Softmax denominators
   accumulate in one shared PSUM bank via PE column groups.
 - A unified filler queue (q-proj chains for block b+1, deferred
   out-projection groups of completed blocks) is drained at a uniform
   credit rate inside every attention iteration, keeping the PE dense.
 - hs tiles for block b+1 prefetch in 8 chunked DMAs at block start;
   output stores are batched to [128, HID] staging tiles and issued on
   the gpsimd queue so the sync queue never blocks input prefetches.
"""

import math
from collections import deque
from functools import partial
import numpy as np
import ml_dtypes

import concourse.bass as bass
import concourse.bacc as bacc
import concourse.mybir as mybir
from concourse.tile import TileContext
from concourse.bass_utils import run_bass_kernel_spmd

B, S, HID = 1, 2048, 4096
HQ, HKV, D = 32, 8, 128
G = HQ // HKV
OBS, W, SINK = 128, 32, 2
THETA = 500000.0
TOP_FRAC, MID_SPARSITY, LOW_FRAC = 0.05, 0.7, 0.20
K_KEEP = int(math.ceil((1.0 - MID_SPARSITY) * D))
SCALE = 1.0 / math.sqrt(D)

N_CORES = 8
CORE_IDS = list(range(N_CORES))
QB = 512            # query block
NQB = S // QB       # 4
KT = 128            # key tile
NKT_P = HID // KT   # 32 contraction tiles for projections

BF = mybir.dt.bfloat16
FR = mybir.dt.float32r
F32 = mybir.dt.float32
F16 = mybir.dt.float16


def _rope_np(x):
    # x: [H, S, D]
    half = D // 2
    inv = 1.0 / (THETA ** (np.arange(half, dtype=np.float32) / half))
    ang = np.arange(S, dtype=np.float32)[:, None] * inv[None, :]
    cos = np.concatenate([np.cos(ang), np.cos(ang)], -1).astype(np.float32)
    sin = np.concatenate([np.sin(ang), np.sin(ang)], -1).astype(np.float32)
    x1, x2 = x[..., :half], x[..., half:]
    rot = np.concatenate([-x2, x1], -1)
    return x * cos[None] + rot * sin[None]


def _build_program(nkc, jm0):
    """nkc[b]: number of 128-key tiles processed for query block b.
    jm0[b]: first tile index that needs a causal/pad mask for block b."""
    nc = bacc.Bacc()
    L = nkc[NQB - 1] * KT                      # padded compacted key count
    nm = [nkc[b] - jm0[b] for b in range(NQB)]  # masked tiles per block
    moff = [sum(nm[:b]) for b in range(NQB)]
    nm_total = sum(nm)

    hs_T = nc.dram_tensor("hs_T", [HID, S], BF, kind="ExternalInput")
    wq_h = nc.dram_tensor("wq_h", [HID, G * D], BF, kind="ExternalInput")
    ksp_T = nc.dram_tensor("ksp_T", [D, L], BF, kind="ExternalInput")
    vsp_r = nc.dram_tensor("vsp_r", [KT, (L // KT) * D], BF, kind="ExternalInput")
    cos_T = nc.dram_tensor("cos_T", [D, S], F32, kind="ExternalInput")
    ssin_T = nc.dram_tensor("ssin_T", [D, S], F32, kind="ExternalInput")
    masks = nc.dram_tensor("masks", [KT, nm_total * QB], BF, kind="ExternalInput")
    ones_l = nc.dram_tensor("ones_l", [KT, KT], BF, kind="ExternalInput")
    wo_h = nc.dram_tensor("wo_h", [128, G * HID], BF, kind="ExternalInput")
    out_ext = nc.dram_tensor("out", [S, HID], F16, kind="ExternalOutput")

    lp = nc.allow_low_precision(reason="bf16 pipeline is intentional")
    lp.__enter__()
    with TileContext(nc) as tc:
        with (
            tc.tile_pool(name="wq", bufs=1) as wq_pool,
            tc.tile_pool(name="wo", bufs=1) as wo_pool,
            tc.tile_pool(name="kv", bufs=1) as kv_pool,
            tc.tile_pool(name="hst", bufs=1) as hs_pool,
            tc.tile_pool(name="qt", bufs=2) as q_pool,
            tc.tile_pool(name="oscp", bufs=3) as osc_pool,
            tc.tile_pool(name="ekp", bufs=3) as e_pool,
            tc.tile_pool(name="tmp", bufs=2) as tmp_pool,
            tc.tile_pool(name="stg", bufs=2) as st_pool,
            tc.tile_pool(name="acc", bufs=1, space="PSUM") as acc_pool,
            tc.tile_pool(name="qps", bufs=1, space="PSUM") as qps_pool,
            tc.tile_pool(name="rot", bufs=3, space="PSUM") as rot_pool,
            tc.tile_pool(name="psl", bufs=1, space="PSUM") as l_pool,
        ):
            ksp_sb = kv_pool.tile([D, L], BF)
            vsp_sb = kv_pool.tile([KT, (L // KT) * D], BF)
            masks_sb = kv_pool.tile([KT, nm_total * QB], BF)
            onesl_sb = kv_pool.tile([KT, KT], BF)
            wq_sb = wq_pool.tile([128, NKT_P * G * D], BF)
            wo_sb = wo_pool.tile([128, G * HID], BF)
            cos_bt = {}
            ssin_bt = {}
            hstb = {}
            qT = {}
            osc = {}

            def load_rope_block(b):
                qs = slice(b * QB, (b + 1) * QB)
                cos_bt[b] = q_pool.tile([D, QB], F32, tag="cosb", name=f"cosb{b}")
                ssin_bt[b] = q_pool.tile([D, QB], F32, tag="sinb", name=f"sinb{b}")
                nc.sync.dma_start(out=cos_bt[b], in_=cos_T[:, qs])
                nc.sync.dma_start(out=ssin_bt[b], in_=ssin_T[:, qs])

            def load_wq_chunk(kt0, kpc):
                # kpc contraction tiles per chunk starting at tile kt0
                r0 = kt0 * 128
                src = wq_h[r0:r0 + kpc * 128, :].rearrange('(a p) d -> p a d', a=kpc)
                dst = wq_sb[:, kt0 * G * D:(kt0 + kpc) * G * D]
                dst = dst.rearrange('p (a d) -> p a d', a=kpc)
                nc.sync.dma_start(out=dst, in_=src)

            def alloc_hstb(b):
                hstb[b] = hs_pool.tile([128, NKT_P * QB], BF, tag="hstb",
                                       name=f"hstb{b}")

            def load_hst_chunk(b, kt0, kpc):
                r0 = kt0 * 128
                qs = slice(b * QB, (b + 1) * QB)
                src = hs_T[r0:r0 + kpc * 128, qs].rearrange('(a p) q -> p a q', a=kpc)
                dst = hstb[b][:, kt0 * QB:(kt0 + kpc) * QB]
                dst = dst.rearrange('p (a q) -> p a q', a=kpc)
                nc.sync.dma_start(out=dst, in_=src)

            def load_wo():
                for g in range(G):
                    nc.sync.dma_start(
                        out=wo_sb[:, g * HID:(g + 1) * HID],
                        in_=wo_h[:, g * HID:(g + 1) * HID],
                    )

            # ---------------- emission helpers ----------------
            def emit_qproj_mm(pss, b, g, kt):
                nc.tensor.matmul(
                    out=pss[:],
                    lhsT=wq_sb[:, kt * G * D + g * D: kt * G * D + (g + 1) * D],
                    rhs=hstb[b][:, kt * QB:(kt + 1) * QB],
                    start=(kt == 0),
                    stop=(kt == NKT_P - 1),
                )

            def rope_y1(b, g, pss, tmps):
                y1 = tmp_pool.tile([D, QB], F32, tag="y1")
                nc.vector.tensor_mul(y1[:], pss[:], cos_bt[b][:])
                tmps['y1'] = y1

            def rope_y2(b, g, pss, tmps):
                y2 = tmp_pool.tile([D, QB], F32, tag="y2")
                nc.vector.tensor_mul(y2[0:64, :], pss[64:128, :],
                                     ssin_bt[b][64:128, :])
                nc.vector.tensor_mul(y2[64:128, :], pss[0:64, :],
                                     ssin_bt[b][0:64, :])
                tmps['y2'] = y2

            def rope_add(b, g, tmps):
                qt = q_pool.tile([D, QB], BF, tag=f"qt{g}", name=f"qt{b}_{g}")
                nc.vector.tensor_add(qt[:], tmps['y1'][:], tmps['y2'][:])
                qT[(b, g)] = qt

            def emit_s_exp_mask(b, kt, g):
                ps_s = rot_pool.tile([KT, QB], F32, tag="rot", name=f"pss{b}_{kt}_{g}")
                nc.tensor.matmul(
                    out=ps_s[:],
                    lhsT=ksp_sb[:, kt * KT:(kt + 1) * KT],
                    rhs=qT[(b, g)][:],
                    start=True,
                    stop=True,
                )
                ek = e_pool.tile([KT, QB], BF, tag="ek", name=f"ek{b}_{kt}_{g}")
                nc.scalar.activation(
                    ek[:], ps_s[:],
                    mybir.ActivationFunctionType.Exp, scale=SCALE,
                )
                if kt >= jm0[b]:
                    slot = moff[b] + (kt - jm0[b])
                    nc.vector.tensor_mul(
                        ek[:], ek[:],
                        masks_sb[:, slot * QB:(slot + 1) * QB],
                    )
                return ek

            def emit_l(b, kt, g, ek, ps_l):
                # all-ones [128,128] stationary: every output partition gets
                # sum_k ek[k, q] — denominator AND its broadcast in one
                # full-rate matmul (no column-group pipeline break)
                nc.tensor.matmul(
                    out=ps_l[:],
                    lhsT=onesl_sb[:],
                    rhs=ek[:],
                    start=(kt == 0),
                    stop=(kt == nkc[b] - 1),
                )

            def emit_o(b, kt, g, ek, ps_o):
                nc.tensor.matmul(
                    out=ps_o[:],
                    lhsT=vsp_sb[:, kt * D:(kt + 1) * D],
                    rhs=ek[:],
                    start=(kt == 0),
                    stop=(kt == nkc[b] - 1),
                )

            # ------------- out-projection (deferred groups) -------------
            st_tiles = {}
            st_count = {}
            evac_ctr = [0]

            def emit_op_group(bb, tt, fc):
                key = (bb, tt)
                if key not in st_tiles:
                    st_tiles[key] = st_pool.tile([128, HID], F16, tag="st",
                                                 name=f"st{bb}_{tt}")
                    st_count[key] = 0
                st = st_tiles[key]
                ps = rot_pool.tile([128, QB], F32, tag="rot", name=f"po{bb}_{tt}_{fc}")
                for g in range(G):
                    nc.tensor.matmul(
                        out=ps[:],
                        lhsT=osc[(bb, g)][:, tt * 128:(tt + 1) * 128],
                        rhs=wo_sb[:, g * HID + fc * QB: g * HID + (fc + 1) * QB],
                        start=(g == 0),
                        stop=(g == G - 1),
                    )
                # evac: 2/3 vector, 1/3 scalar (scalar also runs the exps)
                if evac_ctr[0] % 3 == 2:
                    nc.scalar.copy(st[:, fc * QB:(fc + 1) * QB], ps[:])
                else:
                    nc.vector.tensor_scalar_add(st[:, fc * QB:(fc + 1) * QB],
                                                ps[:], 0.0)
                evac_ctr[0] += 1
                st_count[key] += 1
                if st_count[key] == HID // QB:
                    r0 = bb * QB + tt * 128
                    nc.gpsimd.dma_start(out=out_ext[r0:r0 + 128, :], in_=st[:])
                    del st_tiles[key]

            # ---------------- filler queue ----------------
            # unit = (cost, fn, kind); kind 'q' = q-proj chain work with an
            # end-of-block deadline, 'op' = elastic out-projection work
            units = deque()
            carry = [0.0]

            def pump(credits):
                carry[0] += credits
                while units and carry[0] > 1e-9:
                    cost, fn, _ = units.popleft()
                    carry[0] -= cost
                    fn()

            def pump_all():
                while units:
                    units.popleft()[1]()
                carry[0] = 0.0

            def drain_q_units():
                # force-emit all pending deadline units (preserving their
                # relative order); elastic op units stay queued
                rest = [u for u in units if u[2] == 'op']
                for u in units:
                    if u[2] == 'q':
                        u[1]()
                units.clear()
                units.extend(rest)
                carry[0] = 0.0

            def qchain_units(b, g, kts, pss_holder):
                out = []

                def first(kt=kts[0]):
                    if pss_holder.get('t') is None:
                        pss_holder['t'] = qps_pool.tile(
                            [128, QB], F32, tag="qps", name=f"qps{b}_{g}")
                    emit_qproj_mm(pss_holder['t'], b, g, kt)

                out.append((1.0, first, 'q'))
                for kt in kts[1:]:
                    out.append((1.0, partial(
                        lambda kt_: emit_qproj_mm(pss_holder['t'], b, g, kt_),
                        kt), 'q'))
                return out

            def chain_with_rope(b, g, kts):
                holder = {}
                tmps = {}
                us = qchain_units(b, g, kts, holder)
                us.append((2.0, lambda: rope_y1(b, g, holder['t'], tmps), 'q'))
                us.append((2.0, lambda: rope_y2(b, g, holder['t'], tmps), 'q'))
                us.append((2.0, lambda: rope_add(b, g, tmps), 'q'))
                return us

            def weave(a_units, b_units):
                # proportional merge preserving relative order
                ca = sum(u[0] for u in a_units)
                cb = sum(u[0] for u in b_units)
                out = []
                ia = ib = 0
                sa = sb = 0.0
                while ia < len(a_units) or ib < len(b_units):
                    if ib >= len(b_units):
                        out.append(a_units[ia]); sa += a_units[ia][0]; ia += 1
                    elif ia >= len(a_units):
                        out.append(b_units[ib]); sb += b_units[ib][0]; ib += 1
                    elif sa * cb <= sb * ca:
                        out.append(a_units[ia]); sa += a_units[ia][0]; ia += 1
                    else:
                        out.append(b_units[ib]); sb += b_units[ib][0]; ib += 1
                return out

            # ================= phase Q0: block-0 q-projection =============
            accq = {}
            for g in range(3):
                accq[g] = acc_pool.tile([128, QB], F32, tag=f"acc{g}",
                                        name=f"q0ps{g}")
            g3_holder = {}
            g3_rest = [kt for kt in range(NKT_P) if kt % 2 != 0]

            alloc_hstb(0)
            # small chunks first so the PE starts quickly, then bigger ones
            chunk_plan = [1, 1, 1, 1] + [2] * 14
            side = {
                4: lambda: load_rope_block(0),
                6: lambda: (nc.sync.dma_start(out=onesl_sb, in_=ones_l[:]),
                            nc.sync.dma_start(out=ksp_sb, in_=ksp_T[:])),
                12: lambda: nc.sync.dma_start(out=masks_sb[:, 0:nm[0] * QB],
                                              in_=masks[:, 0:nm[0] * QB]),
                18: lambda: nc.sync.dma_start(out=vsp_sb, in_=vsp_r[:]),
                22: lambda: load_rope_block(1),
                26: lambda: nc.sync.dma_start(out=masks_sb[:, nm[0] * QB:],
                                              in_=masks[:, nm[0] * QB:]),
            }
            kt0 = 0
            for kpc in chunk_plan:
                load_wq_chunk(kt0, kpc)
                load_hst_chunk(0, kt0, kpc)
                if kt0 in side:
                    side[kt0]()
                for a in range(kpc):
                    kt = kt0 + a
                    for g in range(3):
                        emit_qproj_mm(accq[g], 0, g, kt)
                    if kt % 2 == 0:
                        if g3_holder.get('t') is None:
                            g3_holder['t'] = qps_pool.tile(
                                [128, QB], F32, tag="qps", name="qps0_3")
                        emit_qproj_mm(g3_holder['t'], 0, 3, kt)
                kt0 += kpc
            # ropes for g0..2 inline; g3 finishes as filler
            tmps0 = {}
            rope_y1(0, 0, accq[0], tmps0)
            rope_y2(0, 0, accq[0], tmps0)
            rope_add(0, 0, tmps0)

            def mk_q0_g(g):
                tmps = {}
                return [
                    (2.0, lambda: rope_y1(0, g, accq[g], tmps), 'q'),
                    (2.0, lambda: rope_y2(0, g, accq[g], tmps), 'q'),
                    (2.0, lambda: rope_add(0, g, tmps), 'q'),
                ]

            g3_units = qchain_units(0, 3, g3_rest, g3_holder)
            tmps3 = {}
            g3_units.append((2.0, lambda: rope_y1(0, 3, g3_holder['t'], tmps3), 'q'))
            g3_units.append((2.0, lambda: rope_y2(0, 3, g3_holder['t'], tmps3), 'q'))
            g3_units.append((2.0, lambda: rope_add(0, 3, tmps3), 'q'))
            # PE-only g3 mms first (they cover the rope(0,0) latency), ropes
            # for g1/g2 woven in early so attn0's g1/g2 scaffolds are ready
            pre = g3_units[:6] + mk_q0_g(1) + g3_units[6:10] + mk_q0_g(2) \
                + g3_units[10:]
            for u in pre:
                units.append(u)

            # =================== unified attention pipeline ===============
            q_credits = 4 * (NKT_P + 6)
            n_iters_123 = 4 * (nkc[1] + nkc[2] + nkc[3])
            r_rate = (2 * q_credits + 3 * 4 * (QB // 128) * (HID // QB)) \
                / n_iters_123

            for b in range(NQB):
                nkt = nkc[b]
                n_it = 4 * nkt
                if b >= 1:
                    # all chains reading hstb(b) must be emitted before the
                    # hstb(b+1) DMA below reuses the slot (deadlock otherwise);
                    # this also guarantees qT(b,*) exist before the scaffold
                    drain_q_units()
                if b + 1 < NQB:
                    # prefetch next block inputs; queue next q-proj chains
                    alloc_hstb(b + 1)
                    for c in range(8):
                        load_hst_chunk(b + 1, c * (NKT_P // 8), NKT_P // 8)
                    if b == 0:
                        load_wo()
                    if b + 2 < NQB:
                        load_rope_block(b + 2)
                    new_units = []
                    for g in range(G):
                        new_units += chain_with_rope(b + 1, g, list(range(NKT_P)))
                    if b == 0:
                        # keep g3/rope prologue strictly first at block 0
                        units.extend(new_units)
                    else:
                        # weave the chains into only the head of the op queue
                        # so they finish within this block
                        q_cr = sum(u[0] for u in new_units)
                        budget = r_rate * n_it
                        old = list(units)
                        head_cr = max(0.0, budget - q_cr)
                        head = []
                        acc_cr = 0.0
                        while old and acc_cr < head_cr:
                            u = old.pop(0)
                            head.append(u)
                            acc_cr += u[0]
                        merged = weave(new_units, head) + old
                        units.clear()
                        units.extend(merged)
                if b == 0:
                    per_iter = (sum(u[0] for u in units)) / n_it
                else:
                    per_iter = r_rate

                for g in range(G):
                    pso = acc_pool.tile([D, QB], F32, tag=f"acc{g % 3}",
                                        name=f"pso{b}_{g}")
                    ps_l = l_pool.tile([128, QB], F32, tag="psl",
                                       name=f"psl{b}_{g}")
                    prev_ek = None
                    if b == 0 and g == 0:
                        pump(10.0)   # cover rope(0,0) latency with g3/q1 mms
                    for kt in range(nkt):
                        ek = emit_s_exp_mask(b, kt, g)
                        pump(per_iter)
                        if prev_ek is not None:
                            emit_o(b, kt - 1, g, prev_ek, pso)
                            emit_l(b, kt - 1, g, prev_ek, ps_l)
                        prev_ek = ek
                    pump(2.0)
                    emit_o(b, nkt - 1, g, prev_ek, pso)
                    emit_l(b, nkt - 1, g, prev_ek, ps_l)
                    pump(2.0)
                    rsb = tmp_pool.tile([128, QB], F32, tag="rsb")
                    nc.vector.reciprocal_approx_fast(rsb[:], ps_l[:])
                    ot = osc_pool.tile([D, QB], BF, tag=f"osc{g}",
                                       name=f"osc{b}_{g}")
                    nc.vector.tensor_mul(ot[:], pso[:], rsb[:])
                    osc[(b, g)] = ot

                # this block's out-projection becomes filler for later blocks
                for tt in range(QB // 128):
                    for fc in range(HID // QB):
                        units.append((4.0, partial(emit_op_group, b, tt, fc),
                                      'op'))

            pump_all()

    lp.__exit__(None, None, None)
    nc.compile()
    nc.finalize()
    return nc


_NC_CACHE = {}
_LAST_RESULTS = None


def _host_prep(hidden_states, wq, wk, wv):
    hs = hidden_states.reshape(S, HID).astype(np.float32)
    k = (hs @ wk).reshape(S, HKV, D).transpose(1, 0, 2)  # [8, S, D]
    v = (hs @ wv).reshape(S, HKV, D).transpose(1, 0, 2)
    k = _rope_np(k).astype(np.float32)

    obs_q = (hs[S - OBS:] @ wq).reshape(OBS, HQ, D).transpose(1, 0, 2)  # [32, OBS, D]
    half = D // 2
    inv = 1.0 / (THETA ** (np.arange(half, dtype=np.float32) / half))
    ang = np.arange(S - OBS, S)[:, None].astype(np.float32) * inv[None, :]
    cos = np.concatenate([np.cos(ang), np.cos(ang)], -1).astype(np.float32)
    sin = np.concatenate([np.sin(ang), np.sin(ang)], -1).astype(np.float32)
    oq1, oq2 = obs_q[..., :half], obs_q[..., half:]
    obs_q = obs_q * cos[None] + np.concatenate([-oq2, oq1], -1) * sin[None]

    obs_qg = obs_q.reshape(HKV, G, OBS, D)
    s_obs = np.einsum("hgqd,hkd->hgqk", obs_qg, k, optimize=True) * SCALE
    obs_causal = np.arange(S)[None, :] <= (S - OBS + np.arange(OBS))[:, None]
    s_obs = np.where(obs_causal[None, None], s_obs, -np.inf).astype(np.float32)
    m = s_obs.max(-1, keepdims=True)
    e = np.exp(s_obs - m)
    p = e / e.sum(-1, keepdims=True)
    aw = p.astype(np.float32).mean(1)  # [8, OBS, S]
    counts = np.minimum(OBS, S - np.arange(S)).astype(np.float32)
    imp = aw.sum(1) / counts[None, :]  # [8, S]

    imp_c = imp[:, :S - W].reshape(-1)
    t_high = np.quantile(imp_c, 1.0 - TOP_FRAC)
    t_low = np.quantile(imp_c, LOW_FRAC)
    level = np.where(imp >= t_high, 0, np.where(imp < t_low, 2, 1))
    pos = np.arange(S)
    dense = (pos >= S - W) | (pos < SINK)
    level = np.where(dense[None, :], 0, level)

    def topk_mask(x):
        a = np.abs(x)
        thr = np.sort(a, -1)[..., D - K_KEEP]
        return a >= thr[..., None]

    keep_k = np.where((level == 0)[..., None], True, (level == 1)[..., None] & topk_mask(k))
    keep_v = np.where((level == 0)[..., None], True, (level == 1)[..., None] & topk_mask(v))
    k_sp = (k * keep_k).astype(np.float32)
    v_sp = (v * keep_v).astype(np.float32)
    evicted = level == 2  # [8, S]
    return k_sp, v_sp, evicted


def _bf16(x):
    return np.ascontiguousarray(x).astype(ml_dtypes.bfloat16)


def kernel(hidden_states, wq, wk, wv, wo):
    global _LAST_RESULTS

    hs = hidden_states.reshape(S, HID).astype(np.float32)
    k_sp, v_sp, evicted = _host_prep(hidden_states, wq, wk, wv)

    # ---- compact the KV cache: drop evicted keys, keep position order ----
    kept = [np.where(~evicted[h])[0] for h in range(HKV)]
    cle = np.array([[np.searchsorted(kept[h], (b + 1) * QB) for b in range(NQB)]
                    for h in range(HKV)])            # keys with pos < (b+1)*QB
    cl0 = np.array([[np.searchsorted(kept[h], b * QB, side="right") for b in range(NQB)]
                    for h in range(HKV)])            # keys with pos <= b*QB
    nkc = tuple(int(math.ceil(cle[:, b].max() / KT)) for b in range(NQB))
    jm0 = tuple(int(cl0[:, b].min() // KT) for b in range(NQB))
    nm = [nkc[b] - jm0[b] for b in range(NQB)]
    nm_total = sum(nm)
    L = nkc[NQB - 1] * KT

    key = (nkc, jm0)
    if key not in _NC_CACHE:
        _NC_CACHE.clear()
        _NC_CACHE[key] = _build_program(nkc, jm0)
    nc = _NC_CACHE[key]

    hs_T = _bf16(hs.T)
    half = D // 2
    inv = 1.0 / (THETA ** (np.arange(half, dtype=np.float32) / half))
    ang = np.arange(S, dtype=np.float32)[:, None] * inv[None, :]  # [S, 64]
    cosb = np.cos(ang).astype(np.float32)
    sinb = np.sin(ang).astype(np.float32)
    cos_T = np.ascontiguousarray(np.concatenate([cosb, cosb], 1).T)  # [128, S]
    ssin_T = np.ascontiguousarray(np.concatenate([sinb, -sinb], 1).T)  # [128, S]

    in_maps = []
    qq = np.arange(QB)[None, :]
    for h in range(N_CORES):
        idx = kept[h]
        n_kept = len(idx)
        kc = np.zeros((L, D), np.float32)
        vc = np.zeros((L, D), np.float32)
        kc[:n_kept] = k_sp[h][idx]
        vc[:n_kept] = v_sp[h][idx]
        pos_c = np.full(L, 1 << 30, np.int64)
        pos_c[:n_kept] = idx
        # boundary masks: mask[p, q] = pos_c[tile*KT + p] <= b*QB + q
        mk = np.zeros((KT, nm_total * QB), np.float32)
        slot = 0
        for b in range(NQB):
            for j in range(jm0[b], nkc[b]):
                tile_pos = pos_c[j * KT:(j + 1) * KT][:, None]
                mk[:, slot * QB:(slot + 1) * QB] = (tile_pos <= b * QB + qq)
                slot += 1
        vsp_h = vc.reshape(L // KT, KT, D).transpose(1, 0, 2).reshape(KT, (L // KT) * D)
        wo_hh = wo[h * G * D:(h + 1) * G * D, :].reshape(G, 128, HID)
        wo_hh = wo_hh.transpose(1, 0, 2).reshape(128, G * HID)
        in_maps.append({
            "hs_T": hs_T,
            "wq_h": _bf16(wq[:, h * G * D:(h + 1) * G * D]),
            "ksp_T": _bf16(kc.T),
            "vsp_r": _bf16(vsp_h),
            "cos_T": cos_T,
            "ssin_T": ssin_T,
            "masks": _bf16(mk),
            "ones_l": _bf16(np.ones((KT, KT), np.float32)),
            "wo_h": _bf16(wo_hh),
        })

    res = run_bass_kernel_spmd(nc, in_maps, CORE_IDS)
    _LAST_RESULTS = res
    acc = res.results[0]["out"].astype(np.float32)
    for i in range(1, N_CORES):
        acc += res.results[i]["out"].astype(np.float32)
    return acc.reshape(B, S, HID)
